# revision 19
# baseline (speedup 1.0000x reference)
"""COTIC continuous-conv kernel for 8 trn2 NeuronCores (Bass/Tile).

Strategy: data-parallel over batch (2 sequences/core). Per layer:
  Z[l,(h,o)] = featT @ W4r on PE; per-position tap mixing
  out[l,o] = sum_{k,h} h2[l,(k,h)] * Z[l-s_k,h,o] via scalar_tensor_tensor
  (per-partition-scalar FMA) split across DVE + GPSIMD, with the 5 shifted
  Z copies materialized by SBUF->SBUF DMA (DMA can shift partitions; DVE
  cannot). Kernel-MLP h2 computed on PE/ACT in h-major layout, transposed
  to l-major via PE transposes. Sync-BN via tiny per-layer AllReduce.
"""

import numpy as np
import concourse.bass as bass
import concourse.bacc as bacc
import concourse.mybir as mybir
import concourse.tile as tile
from concourse import bass_utils, masks

ALU = mybir.AluOpType
ACTF = mybir.ActivationFunctionType
FP32 = mybir.dt.float32
BF16 = mybir.dt.bfloat16

B, L, CIN0, F = 16, 2048, 64, 128
K, H1, H2, H3 = 5, 8, 16, 32
NL = 4
PAD = 40  # max lag (5 taps * dilation 8)
NC = 8
BLOC = B // NC  # sequences per core
NTILE = L // 128  # l-tiles per sequence
EPS = 1e-5
NTOT = float(B * L)

_cached = {}


def _build_nc():
    nc = bacc.Bacc("TRN2", target_bir_lowering=False, debug=False, num_devices=NC)

    # ---- DRAM I/O ----
    featT0 = nc.dram_tensor("featT0", [BLOC, CIN0, PAD + L], FP32, kind="ExternalInput").ap()
    dt8 = nc.dram_tensor("dt8", [NL, BLOC, K, H1, L], FP32, kind="ExternalInput").ap()
    w4r, wsk, b4m = [], [], []
    mlpp = []
    gam, bet = [], []
    for i in range(NL):
        cin = CIN0 if i == 0 else F
        w4r.append(nc.dram_tensor(f"w4r_{i}", [cin, H3 * F], FP32, kind="ExternalInput").ap())
        wsk.append(nc.dram_tensor(f"wsk_{i}", [cin, F], FP32, kind="ExternalInput").ap())
        b4m.append(nc.dram_tensor(f"b4m_{i}", [cin, F], FP32, kind="ExternalInput").ap())
        # packed small mlp params: [w1(8,1) b1(8,1)] [W2(8,16)] [b2(16,1)] [W3(16,32)] [b3(32,1)]
        mlpp.append({
            "w1": nc.dram_tensor(f"w1_{i}", [H1, 1], FP32, kind="ExternalInput").ap(),
            "b1": nc.dram_tensor(f"b1_{i}", [H1, 1], FP32, kind="ExternalInput").ap(),
            "W2": nc.dram_tensor(f"W2_{i}", [H1, H2], FP32, kind="ExternalInput").ap(),
            "b2": nc.dram_tensor(f"b2_{i}", [H2, 1], FP32, kind="ExternalInput").ap(),
            "W3": nc.dram_tensor(f"W3_{i}", [H2, H3], FP32, kind="ExternalInput").ap(),
            "b3": nc.dram_tensor(f"b3_{i}", [H3, 1], FP32, kind="ExternalInput").ap(),
        })
        gam.append(nc.dram_tensor(f"gam_{i}", [F, 1], FP32, kind="ExternalInput").ap())
        bet.append(nc.dram_tensor(f"bet_{i}", [F, 1], FP32, kind="ExternalInput").ap())
    out_d = nc.dram_tensor("out", [BLOC, L, F], FP32, kind="ExternalOutput").ap()
    upre_d = nc.dram_tensor("upre", [BLOC, NTILE, 128, F], FP32, kind="Internal").ap()
    cc_in = [nc.dram_tensor(f"ccin_{i}", [F, 2], FP32, kind="Internal").ap() for i in range(NL)]
    cc_out = [nc.dram_tensor(f"ccout_{i}", [F, 2], FP32, kind="Internal", addr_space="Shared").ap()
              for i in range(NL)]

    DVE_N = 140  # of the 160 (k,h) slices per tile, first DVE_N on DVE, rest GPSIMD

    with tile.TileContext(nc) as tc:
        import contextlib
        es = contextlib.ExitStack()
        const_p = es.enter_context(tc.tile_pool(name="const", bufs=1))
        ident_f = const_p.tile([128, 128], FP32)
        masks.make_identity(nc, ident_f[:])
        ones_col = const_p.tile([128, 1], FP32)
        nc.vector.memset(ones_col[:], 1.0)
        ones_row = const_p.tile([1, 128], FP32)
        nc.vector.memset(ones_row[:], 1.0)

        # persistent feature buffers (ping/pong), per local sequence
        featA = [const_p.tile([128, PAD + L], FP32, name=f"featA{b}") for b in range(BLOC)]
        featB = [const_p.tile([128, PAD + L], FP32, name=f"featB{b}") for b in range(BLOC)]
        h2sA = [const_p.tile([128, NTILE * K * H3], FP32, name=f"h2sA{b}") for b in range(BLOC)]
        h2sB = [const_p.tile([128, NTILE * K * H3], FP32, name=f"h2sB{b}") for b in range(BLOC)]

        # small per-layer params in SBUF
        sp = es.enter_context(tc.tile_pool(name="smallp", bufs=1))
        s_wsk = [sp.tile([CIN0 if i == 0 else F, F], FP32, name=f"swsk{i}") for i in range(NL)]
        s_b4m = [sp.tile([CIN0 if i == 0 else F, F], FP32, name=f"sb4m{i}") for i in range(NL)]
        s_w1 = [sp.tile([H1, 1], FP32, name=f"sw1{i}") for i in range(NL)]
        s_b1 = [sp.tile([H1, 1], FP32, name=f"sb1{i}") for i in range(NL)]
        s_W2 = [sp.tile([H1, H2], FP32, name=f"sW2{i}") for i in range(NL)]
        s_b2 = [sp.tile([H2, 1], FP32, name=f"sb2{i}") for i in range(NL)]
        s_W3 = [sp.tile([H2, H3], FP32, name=f"sW3{i}") for i in range(NL)]
        s_b3 = [sp.tile([H3, 1], FP32, name=f"sb3{i}") for i in range(NL)]
        s_gam = [sp.tile([F, 1], FP32, name=f"sgam{i}") for i in range(NL)]
        s_bet = [sp.tile([F, 1], FP32, name=f"sbet{i}") for i in range(NL)]
        for i in range(NL):
            nc.sync.dma_start(s_wsk[i][:], wsk[i][:])
            nc.sync.dma_start(s_b4m[i][:], b4m[i][:])
            nc.sync.dma_start(s_w1[i][:], mlpp[i]["w1"])
            nc.sync.dma_start(s_b1[i][:], mlpp[i]["b1"])
            nc.sync.dma_start(s_W2[i][:], mlpp[i]["W2"])
            nc.sync.dma_start(s_b2[i][:], mlpp[i]["b2"])
            nc.sync.dma_start(s_W3[i][:], mlpp[i]["W3"])
            nc.sync.dma_start(s_b3[i][:], mlpp[i]["b3"])
            nc.sync.dma_start(s_gam[i][:], gam[i][:])
            nc.sync.dma_start(s_bet[i][:], bet[i][:])
        for b in range(BLOC):
            nc.vector.memset(featA[b][:, 0:PAD], 0.0)
            nc.sync.dma_start(featA[b][0:CIN0, :], featT0[b])
            nc.vector.memset(featB[b][:, 0:PAD], 0.0)

        # pools
        w4p = es.enter_context(tc.tile_pool(name="w4", bufs=1))
        zsbp = es.enter_context(tc.tile_pool(name="zsb", bufs=2))
        zshp = es.enter_context(tc.tile_pool(name="zsh", bufs=2))
        mlps = es.enter_context(tc.tile_pool(name="mlps", bufs=1))
        scr = es.enter_context(tc.tile_pool(name="scr", bufs=2))
        zp = es.enter_context(tc.tile_pool(name="zp", bufs=1, space="PSUM"))
        skp = es.enter_context(tc.tile_pool(name="skp", bufs=1, space="PSUM"))
        stp = es.enter_context(tc.tile_pool(name="stp", bufs=1, space="PSUM"))
        scp = es.enter_context(tc.tile_pool(name="scp", bufs=2, space="PSUM"))

        def mlp_layer(i, h2s):
            """kernel-MLP for all taps of layer i -> h2s[b][:, T*160 + k*32 + h]"""
            for b in range(BLOC):
                for k in range(K):
                    dt_t = mlps.tile([H1, L], FP32, tag="mlpA")
                    nc.sync.dma_start(dt_t[:], dt8[i, b, k])
                    h1_t = mlps.tile([H1, L], FP32, tag="mlpB")
                    nc.scalar.activation(h1_t[:], dt_t[:], ACTF.Relu,
                                         bias=s_b1[i][:], scale=s_w1[i][:])
                    h2_t = mlps.tile([H2, L], FP32, tag="mlpA")
                    for c in range(L // 512):
                        ps2 = scp.tile([H2, 512], FP32, tag="scp")
                        nc.tensor.matmul(ps2[:], s_W2[i][:], h1_t[:, c * 512:(c + 1) * 512],
                                         start=True, stop=True)
                        nc.scalar.activation(h2_t[:, c * 512:(c + 1) * 512], ps2[:],
                                             ACTF.Relu, bias=s_b2[i][:])
                    h3_t = mlps.tile([H3, L], FP32, tag="mlpB")
                    for c in range(L // 512):
                        ps3 = scp.tile([H3, 512], FP32, tag="scp")
                        nc.tensor.matmul(ps3[:], s_W3[i][:], h2_t[:, c * 512:(c + 1) * 512],
                                         start=True, stop=True)
                        nc.scalar.activation(h3_t[:, c * 512:(c + 1) * 512], ps3[:],
                                             ACTF.Relu, bias=s_b3[i][:])
                    for T in range(NTILE):
                        pst = scp.tile([128, H3], FP32, tag="scp")
                        nc.tensor.transpose(pst[:], h3_t[:, T * 128:(T + 1) * 128],
                                            ident_f[0:H3, 0:H3])
                        nc.vector.tensor_copy(
                            h2s[b][:, T * (K * H3) + k * H3: T * (K * H3) + (k + 1) * H3],
                            pst[:])

        def conv_layer(i, featc, featn, h2s):
            cin = CIN0 if i == 0 else F
            d = 1 << i
            w4_t = w4p.tile([cin, H3 * F], FP32, tag="w4")
            nc.sync.dma_start(w4_t[:], w4r[i][:])
            stats = stp.tile([F, 2], FP32, tag="st")
            first_mm = [True]

            for b in range(BLOC):
                zprev = None
                for T in range(NTILE):
                    base = PAD + T * 128
                    fT = featc[b]
                    # skip connection + b4 terms accumulate in one psum bank
                    psk = skp.tile([128, F], FP32, tag="sk")
                    nc.tensor.matmul(psk[:], fT[0:cin, base:base + 128], s_wsk[i][:],
                                     start=True, stop=False)
                    for k in range(K):
                        s = (K - k) * d
                        nc.tensor.matmul(psk[:], fT[0:cin, base - s:base - s + 128],
                                         s_b4m[i][:], start=False, stop=(k == K - 1))
                    # Z = featT @ W4r  -> [128l, (h,o)], in two 2048-halves
                    zsb = zsbp.tile([128, H3 * F], FP32, tag="zsb")
                    for half in range(2):
                        pz = zp.tile([128, 2048], FP32, tag="z")
                        for c in range(4):
                            off = half * 2048 + c * 512
                            nc.tensor.matmul(pz[:, c * 512:(c + 1) * 512],
                                             fT[0:cin, base:base + 128],
                                             w4_t[:, off:off + 512], start=True, stop=True)
                        for c in range(4):
                            off = half * 2048 + c * 512
                            nc.scalar.copy(zsb[:, off:off + 512], pz[:, c * 512:(c + 1) * 512])
                    # accumulator = skip psum; gpsimd side accumulator
                    acct = scr.tile([128, F], FP32, tag="acc")
                    acc = acct[:]
                    nc.scalar.copy(acc, psk[:])
                    import os
                    if os.environ.get("KDBG") == f"s{i}":
                        nc.sync.dma_start(out_d[b, T * 128:(T + 1) * 128, :], acc)
                    acc2 = None
                    if DVE_N < K * H3:
                        acc2 = scr.tile([128, F], FP32, tag="acc2")
                        nc.gpsimd.memset(acc2[:], 0.0)
                    # tap loop: shifted Z copies via DMA, then per-(k,h) FMA
                    for k in range(K):
                        s = (K - k) * d
                        zk = zshp.tile([128, H3 * F], FP32, tag="zsh")
                        if zprev is not None:
                            nc.sync.dma_start(zk[0:s, :], zprev[128 - s:128, :])
                        else:
                            nc.gpsimd.memset(zk[0:s, :], 0.0)
                        nc.sync.dma_start(zk[s:128, :], zsb[0:128 - s, :])
                        for h in range(H3):
                            idx = k * H3 + h
                            scal = h2s[b][:, T * (K * H3) + idx: T * (K * H3) + idx + 1]
                            if idx < DVE_N:
                                nc.vector.scalar_tensor_tensor(acc, zk[:, h * F:(h + 1) * F],
                                                               scal, acc, ALU.mult, ALU.add)
                            else:
                                # Pool can't run TensorScalarPtr; use TT with a
                                # free-dim-broadcast AP of the per-l scalar.
                                ptmp = scr.tile([128, F], FP32, tag="ptmp")
                                nc.gpsimd.tensor_tensor(ptmp[:], zk[:, h * F:(h + 1) * F],
                                                        scal.broadcast_to([128, F]), ALU.mult)
                                nc.gpsimd.tensor_tensor(acc2[:], acc2[:], ptmp[:], ALU.add)
                    if DVE_N < K * H3:
                        nc.vector.tensor_tensor(acc, acc, acc2[:], ALU.add)
                    import os
                    if os.environ.get("KDBG") == f"u{i}":
                        nc.sync.dma_start(out_d[b, T * 128:(T + 1) * 128, :], acc)
                    zprev = zsb
                    # batch-norm partial sums (over l) via ones-matmul
                    sq = scr.tile([128, F], FP32, tag="sq")
                    nc.gpsimd.tensor_tensor(sq[:], acc, acc, ALU.mult)
                    # only the very first matmul may carry start=True: start
                    # clears has_written for the WHOLE bank, so a second
                    # start=True (other column, same bank) would wipe the
                    # first column's accumulation.
                    nc.tensor.matmul(stats[:, 0:1], acc, ones_col[:],
                                     start=first_mm[0], stop=False)
                    nc.tensor.matmul(stats[:, 1:2], sq[:], ones_col[:],
                                     start=False, stop=(b == BLOC - 1 and T == NTILE - 1))
                    first_mm[0] = False
                    nc.sync.dma_start(upre_d[b, T], acc)

            # sync-BN all-reduce of [F, 2] stats
            st_sb = scr.tile([F, 2], FP32, tag="stsb")
            nc.vector.tensor_copy(st_sb[:], stats[:])
            nc.sync.dma_start(cc_in[i], st_sb[:])
            nc.gpsimd.collective_compute("AllReduce", ALU.add,
                                         replica_groups=[list(range(NC))],
                                         ins=[cc_in[i]], outs=[cc_out[i]])

            # overlap: next layer's kernel-MLP is independent of the collective
            if i + 1 < NL:
                mlp_layer(i + 1, h2sB if i % 2 == 0 else h2sA)

            gst = scr.tile([F, 2], FP32, tag="gst")
            nc.sync.dma_start(gst[:], cc_out[i])
            mu = scr.tile([F, 1], FP32, tag="mu")
            nc.vector.tensor_scalar_mul(mu[:], gst[:, 0:1], 1.0 / NTOT)
            var = scr.tile([F, 1], FP32, tag="var")
            nc.vector.tensor_scalar_mul(var[:], gst[:, 1:2], 1.0 / NTOT)
            mu2 = scr.tile([F, 1], FP32, tag="mu2")
            nc.vector.tensor_tensor(mu2[:], mu[:], mu[:], ALU.mult)
            nc.vector.tensor_tensor(var[:], var[:], mu2[:], ALU.subtract)
            nc.vector.tensor_scalar_add(var[:], var[:], EPS)
            std = scr.tile([F, 1], FP32, tag="std")
            nc.scalar.activation(std[:], var[:], ACTF.Sqrt)
            rstd = scr.tile([F, 1], FP32, tag="rstd")
            nc.vector.reciprocal(rstd[:], std[:])
            ac = scr.tile([F, 2], FP32, tag="ac")
            nc.vector.tensor_tensor(ac[:, 0:1], s_gam[i][:], rstd[:], ALU.mult)  # a
            nc.vector.tensor_tensor(ac[:, 1:2], mu[:], ac[:, 0:1], ALU.mult)    # mu*a
            nc.vector.tensor_tensor(ac[:, 1:2], s_bet[i][:], ac[:, 1:2], ALU.subtract)  # c
            a_bc = scr.tile([128, F], FP32, tag="abc")
            c_bc = scr.tile([128, F], FP32, tag="cbc")
            for col, dst in ((0, a_bc), (1, c_bc)):
                pt = scp.tile([1, 128], FP32, tag="scp")
                nc.tensor.transpose(pt[:], ac[:, col:col + 1], ident_f[:])
                rowt = scr.tile([1, 128], FP32, tag=f"rowt{col}")
                nc.vector.tensor_copy(rowt[:], pt[:])
                pb = scp.tile([128, 128], FP32, tag="scp")
                nc.tensor.matmul(pb[:], ones_row[:], rowt[:], start=True, stop=True)
                nc.vector.tensor_copy(dst[:], pb[:])

            # apply BN + leaky-relu; transpose back to channel-major (or emit output)
            for b in range(BLOC):
                for T in range(NTILE):
                    ut = scr.tile([128, F], FP32, tag="u")
                    nc.sync.dma_start(ut[:], upre_d[b, T])
                    v = scr.tile([128, F], FP32, tag="v")
                    nc.gpsimd.tensor_tensor(v[:], ut[:], a_bc[:], ALU.mult)
                    nc.gpsimd.tensor_tensor(v[:], v[:], c_bc[:], ALU.add)
                    import os
                    if i == NL - 1:
                        fo = scr.tile([128, F], FP32, tag="fo")
                        nc.vector.scalar_tensor_tensor(fo[:], v[:], 0.1, v[:],
                                                       ALU.mult, ALU.max)
                        if not os.environ.get("KDBG"):
                            nc.sync.dma_start(out_d[b, T * 128:(T + 1) * 128, :], fo[:])
                    else:
                        fb = scr.tile([128, F], FP32, tag="fb")
                        nc.vector.scalar_tensor_tensor(fb[:], v[:], 0.1, v[:],
                                                       ALU.mult, ALU.max)
                        if os.environ.get("KDBG") == f"f{i}":
                            nc.sync.dma_start(out_d[b, T * 128:(T + 1) * 128, :], fb[:])
                        ptr = scp.tile([128, 128], FP32, tag="scp")
                        nc.tensor.transpose(ptr[:], fb[:], ident_f[:])
                        nc.vector.tensor_copy(featn[b][:, PAD + T * 128: PAD + (T + 1) * 128],
                                              ptr[:])

        mlp_layer(0, h2sA)
        for i in range(NL):
            featc = featA if i % 2 == 0 else featB
            featn = featB if i % 2 == 0 else featA
            h2s = h2sA if i % 2 == 0 else h2sB
            conv_layer(i, featc, featn, h2s)
        es.close()

    nc.compile()
    return nc


def kernel(event_times, event_types, emb, params):
    event_times = np.asarray(event_times, np.float32)
    event_types = np.asarray(event_types)
    emb = np.asarray(emb, np.float32)

    if "nc" not in _cached:
        _cached["nc"] = _build_nc()
    nc = _cached["nc"]

    # ---- host prep ----
    feat0 = emb[event_types]                      # [B, L, CIN0]
    featT0 = np.zeros((B, CIN0, PAD + L), np.float32)
    featT0[:, :, PAD:] = feat0.transpose(0, 2, 1)

    dt8 = np.zeros((NL, B, K, L), np.float32)
    for i in range(NL):
        d = 1 << i
        for k in range(K):
            s = (K - k) * d
            dt8[i, :, k, s:] = event_times[:, s:] - event_times[:, :L - s]
    dt8 = np.broadcast_to(dt8[:, :, :, None, :], (NL, B, K, H1, L)).copy()

    pw = []
    for i in range(NL):
        p = params[i]
        cin = CIN0 if i == 0 else F
        W4 = np.asarray(p["W4"], np.float32).reshape(H3, cin, F)
        pw.append({
            f"w4r_{i}": np.ascontiguousarray(W4.transpose(1, 0, 2).reshape(cin, H3 * F)),
            f"wsk_{i}": np.asarray(p["Wskip"], np.float32),
            f"b4m_{i}": np.asarray(p["b4"], np.float32).reshape(cin, F),
            f"w1_{i}": np.asarray(p["W1"], np.float32).reshape(1, H1).T.copy(),
            f"b1_{i}": np.asarray(p["b1"], np.float32).reshape(H1, 1),
            f"W2_{i}": np.asarray(p["W2"], np.float32),
            f"b2_{i}": np.asarray(p["b2"], np.float32).reshape(H2, 1),
            f"W3_{i}": np.asarray(p["W3"], np.float32),
            f"b3_{i}": np.asarray(p["b3"], np.float32).reshape(H3, 1),
            f"gam_{i}": np.asarray(p["gamma"], np.float32).reshape(F, 1),
            f"bet_{i}": np.asarray(p["beta"], np.float32).reshape(F, 1),
        })

    in_maps = []
    for c in range(NC):
        m = {
            "featT0": featT0[c * BLOC:(c + 1) * BLOC],
            "dt8": dt8[:, c * BLOC:(c + 1) * BLOC],
        }
        for i in range(NL):
            m.update(pw[i])
        in_maps.append(m)

    last_err = None
    for _attempt in range(3):
        try:
            res = bass_utils.run_bass_kernel_spmd(nc, in_maps, core_ids=list(range(NC)))
            out = np.concatenate([np.asarray(r["out"], np.float32) for r in res.results],
                                 axis=0)
            return out
        except Exception as e:  # transient NRT_EXEC_UNIT_UNRECOVERABLE etc.
            last_err = e
    raise last_err


if __name__ == "__main__":
    pass


# revision 20
# speedup vs baseline: 1.0406x; 1.0406x over previous
"""COTIC continuous-conv kernel for 8 trn2 NeuronCores (Bass/Tile).

Strategy: data-parallel over batch (2 sequences/core). Per layer:
  Z[l,(h,o)] = featT @ W4r on PE; per-position tap mixing
  out[l,o] = sum_{k,h} h2[l,(k,h)] * Z[l-s_k,h,o] via scalar_tensor_tensor
  (per-partition-scalar FMA) split across DVE + GPSIMD, with the 5 shifted
  Z copies materialized by SBUF->SBUF DMA (DMA can shift partitions; DVE
  cannot). Kernel-MLP h2 computed on PE/ACT in h-major layout, transposed
  to l-major via PE transposes. Sync-BN via tiny per-layer AllReduce.
"""

import numpy as np
import concourse.bass as bass
import concourse.bacc as bacc
import concourse.mybir as mybir
import concourse.tile as tile
from concourse import bass_utils, masks

ALU = mybir.AluOpType
ACTF = mybir.ActivationFunctionType
FP32 = mybir.dt.float32
BF16 = mybir.dt.bfloat16

B, L, CIN0, F = 16, 2048, 64, 128
K, H1, H2, H3 = 5, 8, 16, 32
NL = 4
PAD = 40  # max lag (5 taps * dilation 8)
NC = 8
BLOC = B // NC  # sequences per core
NTILE = L // 128  # l-tiles per sequence
EPS = 1e-5
NTOT = float(B * L)

_cached = {}


def _build_nc():
    nc = bacc.Bacc("TRN2", target_bir_lowering=False, debug=False, num_devices=NC)

    # ---- DRAM I/O ----
    featT0 = nc.dram_tensor("featT0", [BLOC, CIN0, PAD + L], FP32, kind="ExternalInput").ap()
    dt8 = nc.dram_tensor("dt8", [NL, BLOC, K, H1, L], FP32, kind="ExternalInput").ap()
    w4r, wsk, b4m = [], [], []
    mlpp = []
    gam, bet = [], []
    for i in range(NL):
        cin = CIN0 if i == 0 else F
        w4r.append(nc.dram_tensor(f"w4r_{i}", [cin, H3 * F], FP32, kind="ExternalInput").ap())
        wsk.append(nc.dram_tensor(f"wsk_{i}", [cin, F], FP32, kind="ExternalInput").ap())
        b4m.append(nc.dram_tensor(f"b4m_{i}", [cin, F], FP32, kind="ExternalInput").ap())
        # packed small mlp params: [w1(8,1) b1(8,1)] [W2(8,16)] [b2(16,1)] [W3(16,32)] [b3(32,1)]
        mlpp.append({
            "w1": nc.dram_tensor(f"w1_{i}", [H1, 1], FP32, kind="ExternalInput").ap(),
            "b1": nc.dram_tensor(f"b1_{i}", [H1, 1], FP32, kind="ExternalInput").ap(),
            "W2": nc.dram_tensor(f"W2_{i}", [H1, H2], FP32, kind="ExternalInput").ap(),
            "b2": nc.dram_tensor(f"b2_{i}", [H2, 1], FP32, kind="ExternalInput").ap(),
            "W3": nc.dram_tensor(f"W3_{i}", [H2, H3], FP32, kind="ExternalInput").ap(),
            "b3": nc.dram_tensor(f"b3_{i}", [H3, 1], FP32, kind="ExternalInput").ap(),
        })
        gam.append(nc.dram_tensor(f"gam_{i}", [F, 1], FP32, kind="ExternalInput").ap())
        bet.append(nc.dram_tensor(f"bet_{i}", [F, 1], FP32, kind="ExternalInput").ap())
    out_d = nc.dram_tensor("out", [BLOC, L, F], FP32, kind="ExternalOutput").ap()
    upre_d = nc.dram_tensor("upre", [BLOC, NTILE, 128, F], FP32, kind="Internal").ap()
    cc_in = [nc.dram_tensor(f"ccin_{i}", [F, 2], FP32, kind="Internal").ap() for i in range(NL)]
    cc_out = [nc.dram_tensor(f"ccout_{i}", [F, 2], FP32, kind="Internal", addr_space="Shared").ap()
              for i in range(NL)]

    DVE_N = 116  # of the 160 (k,h) slices per tile, first DVE_N on DVE, rest GPSIMD

    with tile.TileContext(nc) as tc:
        import contextlib
        es = contextlib.ExitStack()
        const_p = es.enter_context(tc.tile_pool(name="const", bufs=1))
        ident_f = const_p.tile([128, 128], FP32)
        masks.make_identity(nc, ident_f[:])
        ones_col = const_p.tile([128, 1], FP32)
        nc.vector.memset(ones_col[:], 1.0)
        ones_row = const_p.tile([1, 128], FP32)
        nc.vector.memset(ones_row[:], 1.0)

        # persistent feature buffers (ping/pong), per local sequence
        featA = [const_p.tile([128, PAD + L], FP32, name=f"featA{b}") for b in range(BLOC)]
        featB = [const_p.tile([128, PAD + L], FP32, name=f"featB{b}") for b in range(BLOC)]
        h2sA = [const_p.tile([128, NTILE * K * H3], FP32, name=f"h2sA{b}") for b in range(BLOC)]
        h2sB = [const_p.tile([128, NTILE * K * H3], FP32, name=f"h2sB{b}") for b in range(BLOC)]

        # small per-layer params in SBUF
        sp = es.enter_context(tc.tile_pool(name="smallp", bufs=1))
        s_wsk = [sp.tile([CIN0 if i == 0 else F, F], FP32, name=f"swsk{i}") for i in range(NL)]
        s_b4m = [sp.tile([CIN0 if i == 0 else F, F], FP32, name=f"sb4m{i}") for i in range(NL)]
        s_w1 = [sp.tile([H1, 1], FP32, name=f"sw1{i}") for i in range(NL)]
        s_b1 = [sp.tile([H1, 1], FP32, name=f"sb1{i}") for i in range(NL)]
        s_W2 = [sp.tile([H1, H2], FP32, name=f"sW2{i}") for i in range(NL)]
        s_b2 = [sp.tile([H2, 1], FP32, name=f"sb2{i}") for i in range(NL)]
        s_W3 = [sp.tile([H2, H3], FP32, name=f"sW3{i}") for i in range(NL)]
        s_b3 = [sp.tile([H3, 1], FP32, name=f"sb3{i}") for i in range(NL)]
        s_gam = [sp.tile([F, 1], FP32, name=f"sgam{i}") for i in range(NL)]
        s_bet = [sp.tile([F, 1], FP32, name=f"sbet{i}") for i in range(NL)]
        for i in range(NL):
            nc.sync.dma_start(s_wsk[i][:], wsk[i][:])
            nc.sync.dma_start(s_b4m[i][:], b4m[i][:])
            nc.sync.dma_start(s_w1[i][:], mlpp[i]["w1"])
            nc.sync.dma_start(s_b1[i][:], mlpp[i]["b1"])
            nc.sync.dma_start(s_W2[i][:], mlpp[i]["W2"])
            nc.sync.dma_start(s_b2[i][:], mlpp[i]["b2"])
            nc.sync.dma_start(s_W3[i][:], mlpp[i]["W3"])
            nc.sync.dma_start(s_b3[i][:], mlpp[i]["b3"])
            nc.sync.dma_start(s_gam[i][:], gam[i][:])
            nc.sync.dma_start(s_bet[i][:], bet[i][:])
        for b in range(BLOC):
            nc.vector.memset(featA[b][:, 0:PAD], 0.0)
            nc.sync.dma_start(featA[b][0:CIN0, :], featT0[b])
            nc.vector.memset(featB[b][:, 0:PAD], 0.0)

        # pools
        w4p = es.enter_context(tc.tile_pool(name="w4", bufs=1))
        zsbp = es.enter_context(tc.tile_pool(name="zsb", bufs=2))
        zshp = es.enter_context(tc.tile_pool(name="zsh", bufs=2))
        mlps = es.enter_context(tc.tile_pool(name="mlps", bufs=1))
        scr = es.enter_context(tc.tile_pool(name="scr", bufs=2))
        zp = es.enter_context(tc.tile_pool(name="zp", bufs=1, space="PSUM"))
        skp = es.enter_context(tc.tile_pool(name="skp", bufs=1, space="PSUM"))
        stp = es.enter_context(tc.tile_pool(name="stp", bufs=1, space="PSUM"))
        scp = es.enter_context(tc.tile_pool(name="scp", bufs=2, space="PSUM"))

        def mlp_layer(i, h2s):
            """kernel-MLP for all taps of layer i -> h2s[b][:, T*160 + k*32 + h]"""
            for b in range(BLOC):
                for k in range(K):
                    dt_t = mlps.tile([H1, L], FP32, tag="mlpA")
                    nc.sync.dma_start(dt_t[:], dt8[i, b, k])
                    h1_t = mlps.tile([H1, L], FP32, tag="mlpB")
                    nc.scalar.activation(h1_t[:], dt_t[:], ACTF.Relu,
                                         bias=s_b1[i][:], scale=s_w1[i][:])
                    h2_t = mlps.tile([H2, L], FP32, tag="mlpA")
                    for c in range(L // 512):
                        ps2 = scp.tile([H2, 512], FP32, tag="scp")
                        nc.tensor.matmul(ps2[:], s_W2[i][:], h1_t[:, c * 512:(c + 1) * 512],
                                         start=True, stop=True)
                        nc.scalar.activation(h2_t[:, c * 512:(c + 1) * 512], ps2[:],
                                             ACTF.Relu, bias=s_b2[i][:])
                    h3_t = mlps.tile([H3, L], FP32, tag="mlpB")
                    for c in range(L // 512):
                        ps3 = scp.tile([H3, 512], FP32, tag="scp")
                        nc.tensor.matmul(ps3[:], s_W3[i][:], h2_t[:, c * 512:(c + 1) * 512],
                                         start=True, stop=True)
                        nc.scalar.activation(h3_t[:, c * 512:(c + 1) * 512], ps3[:],
                                             ACTF.Relu, bias=s_b3[i][:])
                    for T in range(NTILE):
                        pst = scp.tile([128, H3], FP32, tag="scp")
                        nc.tensor.transpose(pst[:], h3_t[:, T * 128:(T + 1) * 128],
                                            ident_f[0:H3, 0:H3])
                        nc.vector.tensor_copy(
                            h2s[b][:, T * (K * H3) + k * H3: T * (K * H3) + (k + 1) * H3],
                            pst[:])

        def conv_layer(i, featc, featn, h2s):
            cin = CIN0 if i == 0 else F
            d = 1 << i
            w4_t = w4p.tile([cin, H3 * F], FP32, tag="w4")
            nc.sync.dma_start(w4_t[:], w4r[i][:])
            stats = stp.tile([F, 2], FP32, tag="st")
            first_mm = [True]

            for b in range(BLOC):
                zprev = None
                for T in range(NTILE):
                    base = PAD + T * 128
                    fT = featc[b]
                    # skip connection + b4 terms accumulate in one psum bank
                    psk = skp.tile([128, F], FP32, tag="sk")
                    nc.tensor.matmul(psk[:], fT[0:cin, base:base + 128], s_wsk[i][:],
                                     start=True, stop=False)
                    for k in range(K):
                        s = (K - k) * d
                        nc.tensor.matmul(psk[:], fT[0:cin, base - s:base - s + 128],
                                         s_b4m[i][:], start=False, stop=(k == K - 1))
                    # Z = featT @ W4r  -> [128l, (h,o)], in two 2048-halves
                    zsb = zsbp.tile([128, H3 * F], FP32, tag="zsb")
                    for half in range(2):
                        pz = zp.tile([128, 2048], FP32, tag="z")
                        for c in range(4):
                            off = half * 2048 + c * 512
                            nc.tensor.matmul(pz[:, c * 512:(c + 1) * 512],
                                             fT[0:cin, base:base + 128],
                                             w4_t[:, off:off + 512], start=True, stop=True)
                        for c in range(4):
                            off = half * 2048 + c * 512
                            nc.scalar.copy(zsb[:, off:off + 512], pz[:, c * 512:(c + 1) * 512])
                    # accumulator = skip psum; gpsimd side accumulator
                    acct = scr.tile([128, F], FP32, tag="acc")
                    acc = acct[:]
                    nc.scalar.copy(acc, psk[:])
                    import os
                    if os.environ.get("KDBG") == f"s{i}":
                        nc.sync.dma_start(out_d[b, T * 128:(T + 1) * 128, :], acc)
                    acc2 = None
                    if DVE_N < K * H3:
                        acc2 = scr.tile([128, F], FP32, tag="acc2")
                        nc.gpsimd.memset(acc2[:], 0.0)
                    # tap loop: shifted Z copies via DMA, then per-(k,h) FMA
                    for k in range(K):
                        s = (K - k) * d
                        zk = zshp.tile([128, H3 * F], FP32, tag="zsh")
                        if zprev is not None:
                            nc.sync.dma_start(zk[0:s, :], zprev[128 - s:128, :])
                        else:
                            nc.gpsimd.memset(zk[0:s, :], 0.0)
                        nc.sync.dma_start(zk[s:128, :], zsb[0:128 - s, :])
                        for h in range(H3):
                            idx = k * H3 + h
                            scal = h2s[b][:, T * (K * H3) + idx: T * (K * H3) + idx + 1]
                            if idx < DVE_N:
                                nc.vector.scalar_tensor_tensor(acc, zk[:, h * F:(h + 1) * F],
                                                               scal, acc, ALU.mult, ALU.add)
                            else:
                                # third lane: ScalarE does the per-partition
                                # scale-multiply, Pool only the accumulate
                                # (Pool can't run TensorScalarPtr itself).
                                ptmp = scr.tile([128, F], FP32, tag="ptmp")
                                nc.scalar.activation(ptmp[:], zk[:, h * F:(h + 1) * F],
                                                     ACTF.Copy, scale=scal)
                                nc.gpsimd.tensor_tensor(acc2[:], acc2[:], ptmp[:], ALU.add)
                    if DVE_N < K * H3:
                        nc.vector.tensor_tensor(acc, acc, acc2[:], ALU.add)
                    import os
                    if os.environ.get("KDBG") == f"u{i}":
                        nc.sync.dma_start(out_d[b, T * 128:(T + 1) * 128, :], acc)
                    zprev = zsb
                    # batch-norm partial sums (over l) via ones-matmul
                    sq = scr.tile([128, F], FP32, tag="sq")
                    nc.gpsimd.tensor_tensor(sq[:], acc, acc, ALU.mult)
                    # only the very first matmul may carry start=True: start
                    # clears has_written for the WHOLE bank, so a second
                    # start=True (other column, same bank) would wipe the
                    # first column's accumulation.
                    nc.tensor.matmul(stats[:, 0:1], acc, ones_col[:],
                                     start=first_mm[0], stop=False)
                    nc.tensor.matmul(stats[:, 1:2], sq[:], ones_col[:],
                                     start=False, stop=(b == BLOC - 1 and T == NTILE - 1))
                    first_mm[0] = False
                    nc.sync.dma_start(upre_d[b, T], acc)

            # sync-BN all-reduce of [F, 2] stats
            st_sb = scr.tile([F, 2], FP32, tag="stsb")
            nc.vector.tensor_copy(st_sb[:], stats[:])
            nc.sync.dma_start(cc_in[i], st_sb[:])
            nc.gpsimd.collective_compute("AllReduce", ALU.add,
                                         replica_groups=[list(range(NC))],
                                         ins=[cc_in[i]], outs=[cc_out[i]])

            # overlap: next layer's kernel-MLP is independent of the collective
            if i + 1 < NL:
                mlp_layer(i + 1, h2sB if i % 2 == 0 else h2sA)

            gst = scr.tile([F, 2], FP32, tag="gst")
            nc.sync.dma_start(gst[:], cc_out[i])
            mu = scr.tile([F, 1], FP32, tag="mu")
            nc.vector.tensor_scalar_mul(mu[:], gst[:, 0:1], 1.0 / NTOT)
            var = scr.tile([F, 1], FP32, tag="var")
            nc.vector.tensor_scalar_mul(var[:], gst[:, 1:2], 1.0 / NTOT)
            mu2 = scr.tile([F, 1], FP32, tag="mu2")
            nc.vector.tensor_tensor(mu2[:], mu[:], mu[:], ALU.mult)
            nc.vector.tensor_tensor(var[:], var[:], mu2[:], ALU.subtract)
            nc.vector.tensor_scalar_add(var[:], var[:], EPS)
            std = scr.tile([F, 1], FP32, tag="std")
            nc.scalar.activation(std[:], var[:], ACTF.Sqrt)
            rstd = scr.tile([F, 1], FP32, tag="rstd")
            nc.vector.reciprocal(rstd[:], std[:])
            ac = scr.tile([F, 2], FP32, tag="ac")
            nc.vector.tensor_tensor(ac[:, 0:1], s_gam[i][:], rstd[:], ALU.mult)  # a
            nc.vector.tensor_tensor(ac[:, 1:2], mu[:], ac[:, 0:1], ALU.mult)    # mu*a
            nc.vector.tensor_tensor(ac[:, 1:2], s_bet[i][:], ac[:, 1:2], ALU.subtract)  # c
            a_bc = scr.tile([128, F], FP32, tag="abc")
            c_bc = scr.tile([128, F], FP32, tag="cbc")
            for col, dst in ((0, a_bc), (1, c_bc)):
                pt = scp.tile([1, 128], FP32, tag="scp")
                nc.tensor.transpose(pt[:], ac[:, col:col + 1], ident_f[:])
                rowt = scr.tile([1, 128], FP32, tag=f"rowt{col}")
                nc.vector.tensor_copy(rowt[:], pt[:])
                pb = scp.tile([128, 128], FP32, tag="scp")
                nc.tensor.matmul(pb[:], ones_row[:], rowt[:], start=True, stop=True)
                nc.vector.tensor_copy(dst[:], pb[:])

            # apply BN + leaky-relu; transpose back to channel-major (or emit output)
            for b in range(BLOC):
                for T in range(NTILE):
                    ut = scr.tile([128, F], FP32, tag="u")
                    nc.sync.dma_start(ut[:], upre_d[b, T])
                    v = scr.tile([128, F], FP32, tag="v")
                    nc.gpsimd.tensor_tensor(v[:], ut[:], a_bc[:], ALU.mult)
                    nc.gpsimd.tensor_tensor(v[:], v[:], c_bc[:], ALU.add)
                    import os
                    if i == NL - 1:
                        fo = scr.tile([128, F], FP32, tag="fo")
                        nc.vector.scalar_tensor_tensor(fo[:], v[:], 0.1, v[:],
                                                       ALU.mult, ALU.max)
                        if not os.environ.get("KDBG"):
                            nc.sync.dma_start(out_d[b, T * 128:(T + 1) * 128, :], fo[:])
                    else:
                        fb = scr.tile([128, F], FP32, tag="fb")
                        nc.vector.scalar_tensor_tensor(fb[:], v[:], 0.1, v[:],
                                                       ALU.mult, ALU.max)
                        if os.environ.get("KDBG") == f"f{i}":
                            nc.sync.dma_start(out_d[b, T * 128:(T + 1) * 128, :], fb[:])
                        ptr = scp.tile([128, 128], FP32, tag="scp")
                        nc.tensor.transpose(ptr[:], fb[:], ident_f[:])
                        nc.vector.tensor_copy(featn[b][:, PAD + T * 128: PAD + (T + 1) * 128],
                                              ptr[:])

        mlp_layer(0, h2sA)
        for i in range(NL):
            featc = featA if i % 2 == 0 else featB
            featn = featB if i % 2 == 0 else featA
            h2s = h2sA if i % 2 == 0 else h2sB
            conv_layer(i, featc, featn, h2s)
        es.close()

    nc.compile()
    return nc


def kernel(event_times, event_types, emb, params):
    event_times = np.asarray(event_times, np.float32)
    event_types = np.asarray(event_types)
    emb = np.asarray(emb, np.float32)

    if "nc" not in _cached:
        _cached["nc"] = _build_nc()
    nc = _cached["nc"]

    # ---- host prep ----
    feat0 = emb[event_types]                      # [B, L, CIN0]
    featT0 = np.zeros((B, CIN0, PAD + L), np.float32)
    featT0[:, :, PAD:] = feat0.transpose(0, 2, 1)

    dt8 = np.zeros((NL, B, K, L), np.float32)
    for i in range(NL):
        d = 1 << i
        for k in range(K):
            s = (K - k) * d
            dt8[i, :, k, s:] = event_times[:, s:] - event_times[:, :L - s]
    dt8 = np.broadcast_to(dt8[:, :, :, None, :], (NL, B, K, H1, L)).copy()

    pw = []
    for i in range(NL):
        p = params[i]
        cin = CIN0 if i == 0 else F
        W4 = np.asarray(p["W4"], np.float32).reshape(H3, cin, F)
        pw.append({
            f"w4r_{i}": np.ascontiguousarray(W4.transpose(1, 0, 2).reshape(cin, H3 * F)),
            f"wsk_{i}": np.asarray(p["Wskip"], np.float32),
            f"b4m_{i}": np.asarray(p["b4"], np.float32).reshape(cin, F),
            f"w1_{i}": np.asarray(p["W1"], np.float32).reshape(1, H1).T.copy(),
            f"b1_{i}": np.asarray(p["b1"], np.float32).reshape(H1, 1),
            f"W2_{i}": np.asarray(p["W2"], np.float32),
            f"b2_{i}": np.asarray(p["b2"], np.float32).reshape(H2, 1),
            f"W3_{i}": np.asarray(p["W3"], np.float32),
            f"b3_{i}": np.asarray(p["b3"], np.float32).reshape(H3, 1),
            f"gam_{i}": np.asarray(p["gamma"], np.float32).reshape(F, 1),
            f"bet_{i}": np.asarray(p["beta"], np.float32).reshape(F, 1),
        })

    in_maps = []
    for c in range(NC):
        m = {
            "featT0": featT0[c * BLOC:(c + 1) * BLOC],
            "dt8": dt8[:, c * BLOC:(c + 1) * BLOC],
        }
        for i in range(NL):
            m.update(pw[i])
        in_maps.append(m)

    last_err = None
    for _attempt in range(3):
        try:
            res = bass_utils.run_bass_kernel_spmd(nc, in_maps, core_ids=list(range(NC)))
            out = np.concatenate([np.asarray(r["out"], np.float32) for r in res.results],
                                 axis=0)
            return out
        except Exception as e:  # transient NRT_EXEC_UNIT_UNRECOVERABLE etc.
            last_err = e
    raise last_err


if __name__ == "__main__":
    pass
